# revision 3
# baseline (speedup 1.0000x reference)
"""CenterLoss kernel for Trainium2 (8 NeuronCores, data-parallel over batch).

reference:  mean(clip(rowsum((x - labels @ centers)^2), 1e-12, 1e12))
labels are exact one-hot rows, so labels @ centers is an embedding gather.

v3 design (all-fp8 streams, subtract fused into the gather DMA):
  - host casts x -> f8e3 (e3m4, range +-31, 4-bit mantissa), labels -> f8e4
    (0/1 exact), centers -> NEGATED f8e3; x and labels are PRE-PERMUTED on
    host to [P, NT, ...] so every partition's bytes are contiguous in HBM
    (large DMA descriptors, cheap HWDGE issue).
  - per tile n (128 samples x 2048 feat):
      x tile raw fp8 on the two HWDGE rings (sync + scalar engines)
      idx[n]  = max_index(labels[n] == 1.0)       (DVE FIND, f8e4)
      gather-accum: xb[n] += cneg[idx[n]]         (SWDGE indirect DMA,
                                                   compute_op=add -> d = x-c)
      ps[n]   = rowsum(d*d) on ACT (Square+accum_out) or DVE
                (affine_mul_reduce), both run fp8 directly
  - HBM traffic/core ~4.8MB (vs 9.5 bf16); the pacing item is the SWDGE
    descriptor emission for the 8 gather-accums (~1.75us each).

Per-core output is a [128, 11] f32 tile of per-sample sums (tiles 0..6 in
cols 0..6, tile 7 split into 4 quarter-sums in cols 7..10 for a short
tail); the host merges, applies the clip + mean.
"""

import numpy as np
import ml_dtypes

import concourse.bacc as bacc
import concourse.bass as bass
import concourse.mybir as mybir
from concourse.tile import TileContext
from concourse.bass_utils import run_bass_kernel_spmd

F32 = mybir.dt.float32
F8E3 = mybir.dt.float8e3
F8E4 = mybir.dt.float8e4
U32 = mybir.dt.uint32
NP_F8E3 = ml_dtypes.float8_e3m4
NP_F8E4 = ml_dtypes.float8_e4m3

NCORES = 8
B = 8192          # full batch
C = 751           # num classes
D = 2048          # feature dim
BS = B // NCORES  # batch per core = 1024
P = 128           # partitions
NT = BS // P      # batch tiles per core = 8
NQ = 2            # last tile split into halves for a short tail
NACC = NT - 1 + NQ

CLIP_LO, CLIP_HI = 1e-12, 1e12

# engine for each full tile 0..6; DVE also runs the 8 FINDs early, so ACT
# takes the early tiles and DVE picks up late ones once FINDs are done.
SQ_ENGINE = ["act", "act", "act", "dve", "act", "dve", "act"]


def build_nc():
    nc = bacc.Bacc(
        "TRN2",
        target_bir_lowering=False,
        debug=False,
        num_devices=NCORES,
        num_swdge_queues=2,
    )
    # host-permuted: x[p, n, :] = sample (n*P + p); same for labels
    x = nc.dram_tensor("x", [P, NT, D], F8E3, kind="ExternalInput")
    labels = nc.dram_tensor("labels", [P, NT, C], F8E4, kind="ExternalInput")
    cneg = nc.dram_tensor("cneg", [C, D], F8E3, kind="ExternalInput")
    out = nc.dram_tensor("out", [P, NACC], F32, kind="ExternalOutput")

    with TileContext(nc) as tc:
        with tc.tile_pool(name="big", bufs=1) as pool:
            ones = pool.tile([P, 8], F8E4)
            idxs = pool.tile([P, NT, 8], U32)
            acc = pool.tile([P, NACC], F32)
            jdve = pool.tile([P, D], F8E3)
            jact = pool.tile([P, D], F8E3)
            lbig = pool.tile([P, NT, C], F8E4)
            xbig = pool.tile([P, NT, D], F8E3)

            nc.vector.memset(ones[:], 1.0)

            # Early window carries ONLY the two items that gate the gather
            # chain (x tile 0 and the first label tiles); everything else is
            # staged behind them so their completions aren't delayed by
            # round-robin with bulk traffic.
            # sync (qSP) ring: labels[0:2] gates FIND0; x tile 0 follows.
            # The out store rides this ring later.
            nc.sync.dma_start(out=lbig[:, 0:2, :], in_=labels[:, 0:2, :])
            # scalar (qAct) ring: the remaining labels only.
            nc.scalar.dma_start(out=lbig[:, 2:5, :], in_=labels[:, 2:5, :])
            nc.scalar.dma_start(out=lbig[:, 5:8, :], in_=labels[:, 5:8, :])
            # x tiles 0-3 ride SWDGE queue 0 AHEAD of the gathers; tiles
            # 4-7's x is deferred into mid-chain slots on SWDGE queue 1
            # (emitted between gathers, draining on the second queue) so
            # the first gathers' data isn't queued behind 1MB of late x.
            nc.gpsimd.dma_start(out=xbig[:, 0:1, :], in_=x[:, 0:1, :])
            nc.gpsimd.dma_start(out=xbig[:, 1:2, :], in_=x[:, 1:2, :])
            nc.gpsimd.dma_start(out=xbig[:, 2:3, :], in_=x[:, 2:3, :])
            nc.gpsimd.dma_start(out=xbig[:, 3:4, :], in_=x[:, 3:4, :])

            def find(n):
                nc.vector.max_index(
                    out=idxs[:, n, :], in_max=ones[:], in_values=lbig[:, n, :]
                )

            def gather(n):
                # alternate the gather-accums between the two SWDGE queues
                # so their drain/completion phases overlap across queues
                inst = nc.gpsimd.indirect_dma_start(
                    out=xbig[:, n, :],
                    out_offset=None,
                    in_=cneg[:],
                    in_offset=bass.IndirectOffsetOnAxis(
                        ap=idxs[:, n, 0:1], axis=0
                    ),
                    compute_op=mybir.AluOpType.add,
                )
                return inst

            def load_x_q1(lo, hi):
                inst = nc.gpsimd.dma_start(
                    out=xbig[:, lo:hi, :], in_=x[:, lo:hi, :]
                )
                inst.ins.queue = "qPoolDynamic1"

            for n in range(NT):
                find(n)
                gather(n)
                if n == 1:
                    load_x_q1(4, 6)
                if n == 3:
                    load_x_q1(6, 8)

            def square_accum(d_ap, width, acc_col, eng):
                if eng == "act":
                    nc.scalar.activation(
                        out=jact[:, 0:width],
                        in_=d_ap,
                        func=mybir.ActivationFunctionType.Square,
                        accum_out=acc[:, acc_col:acc_col + 1],
                    )
                else:
                    nc.vector.affine_mul_reduce(
                        out=jdve[:, 0:width],
                        accum_out=acc[:, acc_col:acc_col + 1],
                        in0=d_ap,
                        in1=d_ap,
                        scale=1.0,
                        bias=0.0,
                    )

            for n in range(NT - 1):
                square_accum(xbig[:, n, :], D, n, SQ_ENGINE[n])

            # tile 7 in halves, one per engine, for a short parallel tail
            Q = D // NQ
            for q in range(NQ):
                sl = slice(q * Q, (q + 1) * Q)
                square_accum(
                    xbig[:, NT - 1, sl], Q, NT - 1 + q,
                    "act" if q % 2 == 0 else "dve",
                )

            # split store: bulk columns leave while the tail computes, so
            # only the tiny second store's completion sits before the
            # postamble barrier.
            nc.sync.dma_start(out=out[:, 0:NT - 1], in_=acc[:, 0:NT - 1])
            nc.sync.dma_start(out=out[:, NT - 1:], in_=acc[:, NT - 1:])

    nc.compile()
    return nc


_NC = None


def _get_nc():
    global _NC
    if _NC is None:
        _NC = build_nc()
    return _NC


def _shard(inputs: dict):
    x = np.asarray(inputs["x"]).astype(NP_F8E3)
    labels = np.asarray(inputs["labels"]).astype(NP_F8E4)
    cneg = np.ascontiguousarray((-np.asarray(inputs["centers"])).astype(NP_F8E3))
    assert x.shape == (B, D) and labels.shape == (B, C) and cneg.shape == (C, D)
    maps = []
    for k in range(NCORES):
        xk = x[k * BS:(k + 1) * BS].reshape(NT, P, D).transpose(1, 0, 2)
        lk = labels[k * BS:(k + 1) * BS].reshape(NT, P, C).transpose(1, 0, 2)
        maps.append(
            {
                "x": np.ascontiguousarray(xk),
                "labels": np.ascontiguousarray(lk),
                "cneg": cneg,
            }
        )
    return maps


def run_sharded(inputs: dict, trace: bool = False):
    """Shard, run on 8 cores, return (per_sample [B] f32, BassKernelResults)."""
    in_maps = _shard(inputs)
    res = run_bass_kernel_spmd(
        _get_nc(), in_maps, core_ids=list(range(NCORES)), trace=trace
    )
    # out[p, n] holds sample k*BS + n*P + p; cols NT-1.. are the NQ
    # quarter-sums of the last tile
    def merge(o):
        t7 = o[:, NT - 1:].sum(axis=1, keepdims=True)
        return np.concatenate([o[:, :NT - 1], t7], axis=1)

    per_sample = np.concatenate(
        [merge(res.results[k]["out"]).T.reshape(-1) for k in range(NCORES)]
    )
    return per_sample, res


def kernel(x, labels, centers):
    per_sample, _ = run_sharded({"x": x, "labels": labels, "centers": centers})
    per_sample = np.clip(per_sample, CLIP_LO, CLIP_HI)
    return np.asarray(per_sample.mean(dtype=np.float64), dtype=np.float32)


# revision 4
# speedup vs baseline: 1.0202x; 1.0202x over previous
"""CenterLoss kernel for Trainium2 (8 NeuronCores, data-parallel over batch).

reference:  mean(clip(rowsum((x - labels @ centers)^2), 1e-12, 1e12))
labels are exact one-hot rows, so labels @ centers is an embedding gather.

v3 design (all-fp8 streams, subtract fused into the gather DMA):
  - host casts x -> f8e3 (e3m4, range +-31, 4-bit mantissa), labels -> f8e4
    (0/1 exact), centers -> NEGATED f8e3; x and labels are PRE-PERMUTED on
    host to [P, NT, ...] so every partition's bytes are contiguous in HBM
    (large DMA descriptors, cheap HWDGE issue).
  - per tile n (128 samples x 2048 feat):
      x tile raw fp8 on the two HWDGE rings (sync + scalar engines)
      idx[n]  = max_index(labels[n] == 1.0)       (DVE FIND, f8e4)
      gather-accum: xb[n] += cneg[idx[n]]         (SWDGE indirect DMA,
                                                   compute_op=add -> d = x-c)
      ps[n]   = rowsum(d*d) on ACT (Square+accum_out) or DVE
                (affine_mul_reduce), both run fp8 directly
  - HBM traffic/core ~4.8MB (vs 9.5 bf16); the pacing item is the SWDGE
    descriptor emission for the 8 gather-accums (~1.75us each).

Per-core output is a [128, 11] f32 tile of per-sample sums (tiles 0..6 in
cols 0..6, tile 7 split into 4 quarter-sums in cols 7..10 for a short
tail); the host merges, applies the clip + mean.
"""

import numpy as np
import ml_dtypes

import concourse.bacc as bacc
import concourse.bass as bass
import concourse.mybir as mybir
from concourse.tile import TileContext
from concourse.bass_utils import run_bass_kernel_spmd

F32 = mybir.dt.float32
F8E3 = mybir.dt.float8e3
F8E4 = mybir.dt.float8e4
U32 = mybir.dt.uint32
NP_F8E3 = ml_dtypes.float8_e3m4
NP_F8E4 = ml_dtypes.float8_e4m3

NCORES = 8
B = 8192          # full batch
C = 751           # num classes
D = 2048          # feature dim
BS = B // NCORES  # batch per core = 1024
P = 128           # partitions
NT = BS // P      # batch tiles per core = 8
NQ = 2            # last tile split into halves for a short tail
NACC = NT - 1 + NQ

CLIP_LO, CLIP_HI = 1e-12, 1e12

# engine for each full tile 0..6; DVE also runs the 8 FINDs early, so ACT
# takes the early tiles and DVE picks up late ones once FINDs are done.
SQ_ENGINE = ["act", "act", "act", "dve", "act", "dve", "act"]


def build_nc():
    nc = bacc.Bacc(
        "TRN2",
        target_bir_lowering=False,
        debug=False,
        num_devices=NCORES,
        num_swdge_queues=2,
    )
    # host-permuted: x[p, n, :] = sample (n*P + p); same for labels
    x = nc.dram_tensor("x", [P, NT, D], F8E3, kind="ExternalInput")
    labels = nc.dram_tensor("labels", [P, NT, C], F8E4, kind="ExternalInput")
    cneg = nc.dram_tensor("cneg", [C, D], F8E3, kind="ExternalInput")
    out = nc.dram_tensor("out", [P, NACC], F32, kind="ExternalOutput")

    with TileContext(nc) as tc:
        with tc.tile_pool(name="big", bufs=1) as pool:
            ones = pool.tile([P, 8], F8E4)
            idxs = pool.tile([P, NT, 8], U32)
            acc = pool.tile([P, NACC], F32)
            jdve = pool.tile([P, D], F8E3)
            jact = pool.tile([P, D], F8E3)
            lbig = pool.tile([P, NT, C], F8E4)
            xbig = pool.tile([P, NT, D], F8E3)

            nc.vector.memset(ones[:], 1.0)

            # Early window carries ONLY the two items that gate the gather
            # chain (x tile 0 and the first label tiles); everything else is
            # staged behind them so their completions aren't delayed by
            # round-robin with bulk traffic.
            # sync (qSP) ring: labels[0:2] gates FIND0; x tile 0 follows.
            # The out store rides this ring later.
            nc.sync.dma_start(out=lbig[:, 0:1, :], in_=labels[:, 0:1, :])
            # scalar (qAct) ring: the remaining labels only.
            nc.scalar.dma_start(out=lbig[:, 1:4, :], in_=labels[:, 1:4, :])
            nc.scalar.dma_start(out=lbig[:, 4:8, :], in_=labels[:, 4:8, :])
            # x tiles 0-3 ride SWDGE queue 0 AHEAD of the gathers; tiles
            # 4-7's x is deferred into mid-chain slots on SWDGE queue 1
            # (emitted between gathers, draining on the second queue) so
            # the first gathers' data isn't queued behind 1MB of late x.
            nc.gpsimd.dma_start(out=xbig[:, 0:1, :], in_=x[:, 0:1, :])
            nc.gpsimd.dma_start(out=xbig[:, 1:2, :], in_=x[:, 1:2, :])
            nc.gpsimd.dma_start(out=xbig[:, 2:3, :], in_=x[:, 2:3, :])
            nc.gpsimd.dma_start(out=xbig[:, 3:4, :], in_=x[:, 3:4, :])

            def find(n):
                nc.vector.max_index(
                    out=idxs[:, n, :], in_max=ones[:], in_values=lbig[:, n, :]
                )

            def gather(n):
                # alternate the gather-accums between the two SWDGE queues
                # so their drain/completion phases overlap across queues
                inst = nc.gpsimd.indirect_dma_start(
                    out=xbig[:, n, :],
                    out_offset=None,
                    in_=cneg[:],
                    in_offset=bass.IndirectOffsetOnAxis(
                        ap=idxs[:, n, 0:1], axis=0
                    ),
                    compute_op=mybir.AluOpType.add,
                )
                return inst

            def load_x_q1(lo, hi):
                inst = nc.gpsimd.dma_start(
                    out=xbig[:, lo:hi, :], in_=x[:, lo:hi, :]
                )
                inst.ins.queue = "qPoolDynamic1"

            for n in range(NT):
                find(n)
                gather(n)
                if n == 1:
                    load_x_q1(4, 6)
                if n == 3:
                    load_x_q1(6, 8)

            def square_accum(d_ap, width, acc_col, eng):
                if eng == "act":
                    nc.scalar.activation(
                        out=jact[:, 0:width],
                        in_=d_ap,
                        func=mybir.ActivationFunctionType.Square,
                        accum_out=acc[:, acc_col:acc_col + 1],
                    )
                else:
                    nc.vector.affine_mul_reduce(
                        out=jdve[:, 0:width],
                        accum_out=acc[:, acc_col:acc_col + 1],
                        in0=d_ap,
                        in1=d_ap,
                        scale=1.0,
                        bias=0.0,
                    )

            for n in range(NT - 1):
                square_accum(xbig[:, n, :], D, n, SQ_ENGINE[n])

            # tile 7 in halves, one per engine, for a short parallel tail
            Q = D // NQ
            for q in range(NQ):
                sl = slice(q * Q, (q + 1) * Q)
                square_accum(
                    xbig[:, NT - 1, sl], Q, NT - 1 + q,
                    "act" if q % 2 == 0 else "dve",
                )

            # split store: bulk columns leave while the tail computes, so
            # only the tiny second store's completion sits before the
            # postamble barrier.
            nc.sync.dma_start(out=out[:, 0:NT - 1], in_=acc[:, 0:NT - 1])
            nc.sync.dma_start(out=out[:, NT - 1:], in_=acc[:, NT - 1:])

    nc.compile()
    return nc


_NC = None


def _get_nc():
    global _NC
    if _NC is None:
        _NC = build_nc()
    return _NC


def _shard(inputs: dict):
    x = np.asarray(inputs["x"]).astype(NP_F8E3)
    labels = np.asarray(inputs["labels"]).astype(NP_F8E4)
    cneg = np.ascontiguousarray((-np.asarray(inputs["centers"])).astype(NP_F8E3))
    assert x.shape == (B, D) and labels.shape == (B, C) and cneg.shape == (C, D)
    maps = []
    for k in range(NCORES):
        xk = x[k * BS:(k + 1) * BS].reshape(NT, P, D).transpose(1, 0, 2)
        lk = labels[k * BS:(k + 1) * BS].reshape(NT, P, C).transpose(1, 0, 2)
        maps.append(
            {
                "x": np.ascontiguousarray(xk),
                "labels": np.ascontiguousarray(lk),
                "cneg": cneg,
            }
        )
    return maps


def run_sharded(inputs: dict, trace: bool = False):
    """Shard, run on 8 cores, return (per_sample [B] f32, BassKernelResults)."""
    in_maps = _shard(inputs)
    res = run_bass_kernel_spmd(
        _get_nc(), in_maps, core_ids=list(range(NCORES)), trace=trace
    )
    # out[p, n] holds sample k*BS + n*P + p; cols NT-1.. are the NQ
    # quarter-sums of the last tile
    def merge(o):
        t7 = o[:, NT - 1:].sum(axis=1, keepdims=True)
        return np.concatenate([o[:, :NT - 1], t7], axis=1)

    per_sample = np.concatenate(
        [merge(res.results[k]["out"]).T.reshape(-1) for k in range(NCORES)]
    )
    return per_sample, res


def kernel(x, labels, centers):
    per_sample, _ = run_sharded({"x": x, "labels": labels, "centers": centers})
    per_sample = np.clip(per_sample, CLIP_LO, CLIP_HI)
    return np.asarray(per_sample.mean(dtype=np.float64), dtype=np.float32)
